# revision 8
# baseline (speedup 1.0000x reference)
"""DeepClusteringLoss Trainium2 kernel.

loss = (||V^T V||_F^2 - 2 ||V^T E||_F^2 + ||E^T E||_F^2) / (B*N)
summed over batch, with E = embeddings.reshape(B, N, D), V =
assignments.reshape(B, N, S), N = F*T.

Sharding: data-parallel over batch; each of the 8 cores handles one batch
element.  The host pre-interleaves W = [V | E] (N x 44) and pre-casts to a
narrow dtype (fp8e4m3 by default), so the on-chip work is a pure Gram
accumulation G = W^T W streamed through the PE array: per 128-row chunk one
LDWEIGHTS+MATMUL pair (or per 256 rows in fp8 DoubleRow mode), accumulating
in PSUM fp32.  Host casting is mathematically identical to casting on-chip
(the matmul operands are narrow either way) but halves/quarters the
compulsory HBM reads, which is the roofline for this kernel.

All DMAs are HWDGE, issued alternately from the Sync and Scalar queues so
descriptor generation never serializes behind one engine.  There are no
on-chip interleave copies: the DMA'd tile is matmul-ready.  The per-core
scalar partial loss = ||G||^2 - 4 ||B||^2 (B = V^T E block) is reduced
on-device; the host sums the 8 partials and divides by B*N.
"""

import os
from contextlib import ExitStack

import numpy as np
import ml_dtypes

import concourse.bacc as bacc
import concourse.mybir as mybir
import concourse.tile as tile
from concourse.bass_utils import run_bass_kernel_spmd

B, F, T, D, S = 8, 256, 512, 40, 4
N = F * T              # rows per core (131072)
SD = S + D             # 44 combined features
P = 128                # partitions / chunk rows
N_CORES = 8

# MODE: fp16 | fp8 | fp8sw (fp8 with DoubleRowSwInterleave 256-row chunks;
# W padded to 48 columns so the sub-row stride satisfies step%16==0)
MODE = os.environ.get("KERNEL_MODE", "fp8sw")
COL_TILE = os.environ.get("KERNEL_COL_TILE", "0") == "1"
W_BUFS = int(os.environ.get("KERNEL_BUFS", "6"))

# block schedule in 128-row chunks: small blocks first for fast engine
# start-up, small blocks last so the final DMA->matmul drain is short.
BLOCKS = [16, 16, 32, 64] + [64] * 13 + [32, 16, 8, 8]
assert sum(BLOCKS) == N // P

_nc_cache = {}


def _mode_dt(mode):
    if mode == "fp16":
        return mybir.dt.float16, np.float16
    return mybir.dt.float8e4, ml_dtypes.float8_e4m3


def _build_nc(key):
    mode, col_tile, w_bufs = key
    mm_dt, _ = _mode_dt(mode)
    f32 = mybir.dt.float32
    dr = mode == "fp8sw"
    sdp = 48 if dr else SD      # stored W columns (padded for DoubleRow)

    nc = bacc.Bacc("TRN2", target_bir_lowering=False, debug=False)
    W = nc.dram_tensor("w", (N, sdp), mm_dt, kind="ExternalInput")
    OUT = nc.dram_tensor("partial", (1, 1), f32, kind="ExternalOutput")

    with tile.TileContext(nc) as tc, ExitStack() as ctx:
        # Whole W fits in SBUF (44-48 KB/partition in fp8), so every block
        # gets its own persistent tile: matmuls never wait on buffer
        # recycling, and the DMA stream runs purely engine-rate-limited.
        w_pool = ctx.enter_context(tc.tile_pool(name="w", bufs=1))
        psum_pool = ctx.enter_context(tc.tile_pool(name="ps", bufs=1, space="PSUM"))
        # col_tile: even chunks accumulate into partitions [0:SD] (PE col
        # groups 0-1), odd chunks into [64:64+SD] (col groups 2-3)
        g_ps = psum_pool.tile([64 + sdp if col_tile else sdp, sdp], f32, tag="g")

        ep = ctx.enter_context(tc.tile_pool(name="ep", bufs=1))
        ones = ep.tile([sdp, 1], f32, tag="on")
        nc.vector.memset(ones[:], 1.0)

        chunk = 0          # global (possibly double-row) chunk counter
        step = 2 if dr else 1
        n_chunks = sum(BLOCKS) // step
        r0 = 0
        for blk, ub in enumerate(BLOCKS):
            rows = ub * P
            w_ap = W[r0:r0 + rows, :].rearrange("(p u) c -> p (u c)", p=P)
            r0 += rows
            w_t = w_pool.tile([P, ub * sdp], mm_dt, tag=f"w{blk}")
            # rotate across three DMA-issue queues (2x HWDGE + SWDGE) so
            # descriptor generation and queue-ring depth never gate delivery
            eng = (nc.sync, nc.scalar, nc.gpsimd)[blk % 3]
            eng.dma_start(out=w_t[:], in_=w_ap)

            w3 = w_t[:].rearrange("p (u c) -> p u c", c=sdp)
            for u in range(0, ub, step):
                wu = w3[:, u:u + 2, :] if dr else w3[:, u, :]
                kw = (dict(perf_mode=mybir.MatmulPerfMode.DoubleRowSwInterleave)
                      if dr else {})
                if col_tile:
                    half = chunk % 2
                    out_ap = g_ps[64 * half:64 * half + sdp, :]
                    nc.tensor.matmul(
                        out_ap, wu, wu,
                        start=(chunk < 2),
                        stop=(chunk >= n_chunks - 2),
                        tile_position=(0, 64 * half),
                        skip_group_check=True,
                        **kw,
                    )
                else:
                    nc.tensor.matmul(
                        g_ps[:], wu, wu,
                        start=(chunk == 0),
                        stop=(chunk == n_chunks - 1),
                        **kw,
                    )
                chunk += 1

        # Epilogue: partial = sum(G^2) - 4 * sum(B^2), B = G[0:S, S:SD]
        g2 = ep.tile([sdp, sdp], f32, tag="g2")
        g_sb = ep.tile([sdp, sdp], f32, tag="gsb")
        if col_tile:
            # DVE lanes can't read across partition bases, so shift the odd
            # half down with a tiny SBUF->SBUF HWDGE DMA and add the halves.
            o_sb = ep.tile([64 + sdp, sdp], f32, tag="osb")
            nc.vector.tensor_copy(o_sb[64:64 + sdp, :], g_ps[64:64 + sdp, :])
            shifted = ep.tile([sdp, sdp], f32, tag="sh")
            nc.sync.dma_start(out=shifted[:], in_=o_sb[64:64 + sdp, :])
            nc.vector.tensor_add(g_sb[:], g_ps[0:sdp, :], shifted[:])
        else:
            nc.vector.tensor_copy(g_sb[:], g_ps[0:sdp, :])
        nc.vector.tensor_mul(g2[:], g_sb[:], g_sb[:])
        colsum = ep.tile([sdp, 1], f32, tag="cs")
        nc.vector.reduce_sum(colsum[:], g2[:], axis=mybir.AxisListType.X)
        bcol = ep.tile([S, 1], f32, tag="bc")
        nc.vector.reduce_sum(bcol[:], g2[0:S, S:SD], axis=mybir.AxisListType.X)
        bneg = ep.tile([S, 1], f32, tag="bn")
        nc.vector.tensor_scalar_mul(bneg[:], bcol[:], -4.0)
        s_ps = psum_pool.tile([1, 1], f32, tag="s")
        nc.tensor.matmul(s_ps[:], colsum[:], ones[:], start=True, stop=False)
        nc.tensor.matmul(s_ps[:], bneg[:], ones[0:S, :], start=False, stop=True)
        res = ep.tile([1, 1], f32, tag="r")
        nc.vector.tensor_copy(res[:], s_ps[:])
        nc.sync.dma_start(out=OUT[:, :], in_=res[:])

    nc.finalize()
    return nc


def _get_nc():
    key = (MODE, COL_TILE, W_BUFS)
    if key not in _nc_cache:
        _nc_cache[key] = _build_nc(key)
    return _nc_cache[key]


def _host_w(embeddings, assignments):
    _, np_dt = _mode_dt(MODE)
    sdp = 48 if MODE == "fp8sw" else SD
    ws = []
    for i in range(N_CORES):
        w = np.zeros((N, sdp), dtype=np_dt)
        w[:, 0:S] = assignments[i].reshape(N, S).astype(np_dt)
        w[:, S:SD] = embeddings[i].reshape(N, D).astype(np_dt)
        ws.append(w)
    return ws


def _run(embeddings: np.ndarray, assignments: np.ndarray, trace: bool = False):
    nc = _get_nc()
    in_maps = [{"w": w} for w in _host_w(embeddings, assignments)]
    try:
        res = run_bass_kernel_spmd(
            nc, in_maps, core_ids=list(range(N_CORES)), trace=trace
        )
    except Exception:
        # transient NRT/device hiccups (e.g. NRT_EXEC_UNIT_UNRECOVERABLE)
        # have been observed to succeed on retry
        res = run_bass_kernel_spmd(
            nc, in_maps, core_ids=list(range(N_CORES)), trace=trace
        )
    partials = [float(r["partial"][0, 0]) for r in res.results]
    total = np.float32(np.sum(np.asarray(partials, dtype=np.float64)) / (B * N))
    return np.asarray(total, dtype=np.float32), res


def kernel(embeddings: np.ndarray, assignments: np.ndarray) -> np.ndarray:
    out, _ = _run(embeddings, assignments, trace=False)
    return out


# revision 9
# speedup vs baseline: 1.0531x; 1.0531x over previous
"""DeepClusteringLoss Trainium2 kernel.

loss = (||V^T V||_F^2 - 2 ||V^T E||_F^2 + ||E^T E||_F^2) / (B*N)
summed over batch, with E = embeddings.reshape(B, N, D), V =
assignments.reshape(B, N, S), N = F*T.

Sharding: data-parallel over batch; each of the 8 cores handles one batch
element.  The host pre-interleaves W = [V | E] (N x 44) and pre-casts to a
narrow dtype (fp8e4m3 by default), so the on-chip work is a pure Gram
accumulation G = W^T W streamed through the PE array: per 128-row chunk one
LDWEIGHTS+MATMUL pair (or per 256 rows in fp8 DoubleRow mode), accumulating
in PSUM fp32.  Host casting is mathematically identical to casting on-chip
(the matmul operands are narrow either way) but halves/quarters the
compulsory HBM reads, which is the roofline for this kernel.

All DMAs are HWDGE, issued alternately from the Sync and Scalar queues so
descriptor generation never serializes behind one engine.  There are no
on-chip interleave copies: the DMA'd tile is matmul-ready.  The per-core
scalar partial loss = ||G||^2 - 4 ||B||^2 (B = V^T E block) is reduced
on-device; the host sums the 8 partials and divides by B*N.
"""

import os
from contextlib import ExitStack

import numpy as np
import ml_dtypes

import concourse.bacc as bacc
import concourse.mybir as mybir
import concourse.tile as tile
from concourse.bass_utils import run_bass_kernel_spmd

B, F, T, D, S = 8, 256, 512, 40, 4
N = F * T              # rows per core (131072)
SD = S + D             # 44 combined features
P = 128                # partitions / chunk rows
N_CORES = 8

# MODE: fp16 | fp8 | fp8sw (fp8 with DoubleRowSwInterleave 256-row chunks;
# W padded to 48 columns so the sub-row stride satisfies step%16==0)
MODE = os.environ.get("KERNEL_MODE", "fp8sw")
COL_TILE = os.environ.get("KERNEL_COL_TILE", "0") == "1"
W_BUFS = int(os.environ.get("KERNEL_BUFS", "6"))

# block schedule in 128-row chunks: small blocks first for fast engine
# start-up, small blocks last so the final DMA->matmul drain is short.
BLOCKS = [32, 64] + [128] * 6 + [96, 32, 16, 8, 8]
assert sum(BLOCKS) == N // P

_nc_cache = {}


def _mode_dt(mode):
    if mode == "fp16":
        return mybir.dt.float16, np.float16
    return mybir.dt.float8e4, ml_dtypes.float8_e4m3


def _build_nc(key):
    mode, col_tile, w_bufs = key
    mm_dt, _ = _mode_dt(mode)
    f32 = mybir.dt.float32
    dr = mode == "fp8sw"
    sdp = 48 if dr else SD      # stored W columns (padded for DoubleRow)

    nc = bacc.Bacc("TRN2", target_bir_lowering=False, debug=False)
    W = nc.dram_tensor("w", (N, sdp), mm_dt, kind="ExternalInput")
    OUT = nc.dram_tensor("partial", (1, 1), f32, kind="ExternalOutput")

    with tile.TileContext(nc) as tc, ExitStack() as ctx:
        # Whole W fits in SBUF (44-48 KB/partition in fp8), so every block
        # gets its own persistent tile: matmuls never wait on buffer
        # recycling, and the DMA stream runs purely engine-rate-limited.
        w_pool = ctx.enter_context(tc.tile_pool(name="w", bufs=1))
        psum_pool = ctx.enter_context(tc.tile_pool(name="ps", bufs=1, space="PSUM"))
        # col_tile: even chunks accumulate into partitions [0:SD] (PE col
        # groups 0-1), odd chunks into [64:64+SD] (col groups 2-3)
        g_ps = psum_pool.tile([64 + sdp if col_tile else sdp, sdp], f32, tag="g")

        ep = ctx.enter_context(tc.tile_pool(name="ep", bufs=1))
        ones = ep.tile([sdp, 1], f32, tag="on")
        nc.vector.memset(ones[:], 1.0)

        chunk = 0          # global (possibly double-row) chunk counter
        step = 2 if dr else 1
        n_chunks = sum(BLOCKS) // step
        r0 = 0
        for blk, ub in enumerate(BLOCKS):
            rows = ub * P
            w_ap = W[r0:r0 + rows, :].rearrange("(p u) c -> p (u c)", p=P)
            r0 += rows
            w_t = w_pool.tile([P, ub * sdp], mm_dt, tag=f"w{blk}")
            # alternate the two HWDGE queues so descriptor-gen is parallel
            eng = nc.sync if blk % 2 == 0 else nc.scalar
            eng.dma_start(out=w_t[:], in_=w_ap)

            w3 = w_t[:].rearrange("p (u c) -> p u c", c=sdp)
            for u in range(0, ub, step):
                wu = w3[:, u:u + 2, :] if dr else w3[:, u, :]
                kw = (dict(perf_mode=mybir.MatmulPerfMode.DoubleRowSwInterleave)
                      if dr else {})
                if col_tile:
                    half = chunk % 2
                    out_ap = g_ps[64 * half:64 * half + sdp, :]
                    nc.tensor.matmul(
                        out_ap, wu, wu,
                        start=(chunk < 2),
                        stop=(chunk >= n_chunks - 2),
                        tile_position=(0, 64 * half),
                        skip_group_check=True,
                        **kw,
                    )
                else:
                    nc.tensor.matmul(
                        g_ps[:], wu, wu,
                        start=(chunk == 0),
                        stop=(chunk == n_chunks - 1),
                        **kw,
                    )
                chunk += 1

        # Epilogue: partial = sum(G^2) - 4 * sum(B^2), B = G[0:S, S:SD]
        g2 = ep.tile([sdp, sdp], f32, tag="g2")
        g_sb = ep.tile([sdp, sdp], f32, tag="gsb")
        if col_tile:
            # DVE lanes can't read across partition bases, so shift the odd
            # half down with a tiny SBUF->SBUF HWDGE DMA and add the halves.
            o_sb = ep.tile([64 + sdp, sdp], f32, tag="osb")
            nc.vector.tensor_copy(o_sb[64:64 + sdp, :], g_ps[64:64 + sdp, :])
            shifted = ep.tile([sdp, sdp], f32, tag="sh")
            nc.sync.dma_start(out=shifted[:], in_=o_sb[64:64 + sdp, :])
            nc.vector.tensor_add(g_sb[:], g_ps[0:sdp, :], shifted[:])
        else:
            nc.vector.tensor_copy(g_sb[:], g_ps[0:sdp, :])
        nc.vector.tensor_mul(g2[:], g_sb[:], g_sb[:])
        colsum = ep.tile([sdp, 1], f32, tag="cs")
        nc.vector.reduce_sum(colsum[:], g2[:], axis=mybir.AxisListType.X)
        bcol = ep.tile([S, 1], f32, tag="bc")
        nc.vector.reduce_sum(bcol[:], g2[0:S, S:SD], axis=mybir.AxisListType.X)
        bneg = ep.tile([S, 1], f32, tag="bn")
        nc.vector.tensor_scalar_mul(bneg[:], bcol[:], -4.0)
        s_ps = psum_pool.tile([1, 1], f32, tag="s")
        nc.tensor.matmul(s_ps[:], colsum[:], ones[:], start=True, stop=False)
        nc.tensor.matmul(s_ps[:], bneg[:], ones[0:S, :], start=False, stop=True)
        res = ep.tile([1, 1], f32, tag="r")
        nc.vector.tensor_copy(res[:], s_ps[:])
        nc.sync.dma_start(out=OUT[:, :], in_=res[:])

    nc.finalize()
    return nc


def _get_nc():
    key = (MODE, COL_TILE, W_BUFS)
    if key not in _nc_cache:
        _nc_cache[key] = _build_nc(key)
    return _nc_cache[key]


def _host_w(embeddings, assignments):
    _, np_dt = _mode_dt(MODE)
    sdp = 48 if MODE == "fp8sw" else SD
    ws = []
    for i in range(N_CORES):
        w = np.zeros((N, sdp), dtype=np_dt)
        w[:, 0:S] = assignments[i].reshape(N, S).astype(np_dt)
        w[:, S:SD] = embeddings[i].reshape(N, D).astype(np_dt)
        ws.append(w)
    return ws


def _run(embeddings: np.ndarray, assignments: np.ndarray, trace: bool = False):
    nc = _get_nc()
    in_maps = [{"w": w} for w in _host_w(embeddings, assignments)]
    try:
        res = run_bass_kernel_spmd(
            nc, in_maps, core_ids=list(range(N_CORES)), trace=trace
        )
    except Exception:
        # transient NRT/device hiccups (e.g. NRT_EXEC_UNIT_UNRECOVERABLE)
        # have been observed to succeed on retry
        res = run_bass_kernel_spmd(
            nc, in_maps, core_ids=list(range(N_CORES)), trace=trace
        )
    partials = [float(r["partial"][0, 0]) for r in res.results]
    total = np.float32(np.sum(np.asarray(partials, dtype=np.float64)) / (B * N))
    return np.asarray(total, dtype=np.float32), res


def kernel(embeddings: np.ndarray, assignments: np.ndarray) -> np.ndarray:
    out, _ = _run(embeddings, assignments, trace=False)
    return out


# revision 12
# speedup vs baseline: 1.0571x; 1.0038x over previous
"""DeepClusteringLoss Trainium2 kernel.

loss = (||V^T V||_F^2 - 2 ||V^T E||_F^2 + ||E^T E||_F^2) / (B*N)
summed over batch, with E = embeddings.reshape(B, N, D), V =
assignments.reshape(B, N, S), N = F*T.

Sharding: data-parallel over batch; each of the 8 cores handles one batch
element.  The host pre-interleaves W = [V | E] (N x 44) and pre-casts to a
narrow dtype (fp8e4m3 by default), so the on-chip work is a pure Gram
accumulation G = W^T W streamed through the PE array: per 128-row chunk one
LDWEIGHTS+MATMUL pair (or per 256 rows in fp8 DoubleRow mode), accumulating
in PSUM fp32.  Host casting is mathematically identical to casting on-chip
(the matmul operands are narrow either way) but halves/quarters the
compulsory HBM reads, which is the roofline for this kernel.

All DMAs are HWDGE, issued alternately from the Sync and Scalar queues so
descriptor generation never serializes behind one engine.  There are no
on-chip interleave copies: the DMA'd tile is matmul-ready.  The per-core
scalar partial loss = ||G||^2 - 4 ||B||^2 (B = V^T E block) is reduced
on-device; the host sums the 8 partials and divides by B*N.
"""

import os
from contextlib import ExitStack

import numpy as np
import ml_dtypes

import concourse.bacc as bacc
import concourse.mybir as mybir
import concourse.tile as tile
from concourse.bass_utils import run_bass_kernel_spmd

B, F, T, D, S = 8, 256, 512, 40, 4
N = F * T              # rows per core (131072)
SD = S + D             # 44 combined features
P = 128                # partitions / chunk rows
N_CORES = 8

# MODE: fp16 | fp8 | fp8fwl
# fp8fwl: stationary spans 128 columns (two 44-col chunks + 40 junk cols)
# so the compiler's Fast Weight Load kicks in (NumWeights==128), loading
# weights at 4 fp8/cycle instead of 1; the junk contributions land in PSUM
# partitions the epilogue never reads.
MODE = os.environ.get("KERNEL_MODE", "fp8fwl")
COL_TILE = os.environ.get("KERNEL_COL_TILE", "0") == "1"
W_BUFS = int(os.environ.get("KERNEL_BUFS", "6"))

# block schedule in 128-row chunks: small blocks first for fast engine
# start-up, small blocks last so the final DMA->matmul drain is short.
BLOCKS = [32, 64] + [128] * 6 + [96, 32, 16, 8, 8]
assert sum(BLOCKS) == N // P

_nc_cache = {}


def _mode_dt(mode):
    if mode == "fp16":
        return mybir.dt.float16, np.float16
    return mybir.dt.float8e4, ml_dtypes.float8_e4m3


def _build_nc(key):
    mode, col_tile, w_bufs = key
    mm_dt, _ = _mode_dt(mode)
    f32 = mybir.dt.float32
    fwl = mode == "fp8fwl"
    sdp = SD

    nc = bacc.Bacc("TRN2", target_bir_lowering=False, debug=False)
    W = nc.dram_tensor("w", (N, sdp), mm_dt, kind="ExternalInput")
    OUT = nc.dram_tensor("partial", (1, 1), f32, kind="ExternalOutput")

    with tile.TileContext(nc) as tc, ExitStack() as ctx:
        # Whole W fits in SBUF (44-48 KB/partition in fp8), so every block
        # gets its own persistent tile: matmuls never wait on buffer
        # recycling, and the DMA stream runs purely engine-rate-limited.
        w_pool = ctx.enter_context(tc.tile_pool(name="w", bufs=1))
        psum_pool = ctx.enter_context(tc.tile_pool(name="ps", bufs=1, space="PSUM"))
        # col_tile: even chunks accumulate into partitions [0:SD] (PE col
        # groups 0-1), odd chunks into [64:64+SD] (col groups 2-3)
        g_ps = psum_pool.tile([64 + sdp if col_tile else sdp, sdp], f32, tag="g")

        ep = ctx.enter_context(tc.tile_pool(name="ep", bufs=1))
        ones = ep.tile([sdp, 1], f32, tag="on")
        nc.vector.memset(ones[:], 1.0)

        if fwl:
            # two PSUM accumulators: gA rows [0:44] collect even-chunk Grams,
            # gB rows [44:88] collect odd-chunk Grams (plus junk elsewhere)
            ga_ps = psum_pool.tile([P, SD], f32, tag="ga")
            gb_ps = psum_pool.tile([P, SD], f32, tag="gb")

        chunk = 0          # global chunk counter
        n_chunks = sum(BLOCKS)
        n_pairs = n_chunks // 2
        pair = 0
        r0 = 0
        for blk, ub in enumerate(BLOCKS):
            rows = ub * P
            w_ap = W[r0:r0 + rows, :].rearrange("(p u) c -> p (u c)", p=P)
            r0 += rows
            w_t = w_pool.tile([P, ub * sdp], mm_dt, tag=f"w{blk}")
            # alternate the two HWDGE queues so descriptor-gen is parallel
            eng = nc.sync if blk % 2 == 0 else nc.scalar
            eng.dma_start(out=w_t[:], in_=w_ap)

            w3 = w_t[:].rearrange("p (u c) -> p u c", c=sdp)
            if fwl:
                for u in range(0, ub, 2):
                    # 128-col stationary (FWL) except the last pair of each
                    # block, whose junk cols would read past the tile
                    wcols = 2 * sdp if u == ub - 2 else 128
                    wst = w_t[0:P, u * sdp:u * sdp + wcols]
                    nc.tensor.matmul(
                        ga_ps[0:wcols, :], wst, w3[:, u, :],
                        start=(pair == 0), stop=(pair == n_pairs - 1),
                        skip_group_check=True,
                    )
                    nc.tensor.matmul(
                        gb_ps[0:wcols, :], wst, w3[:, u + 1, :],
                        start=(pair == 0), stop=(pair == n_pairs - 1),
                        skip_group_check=True,
                    )
                    pair += 1
                continue
            for u in range(ub):
                wu = w3[:, u, :]
                if col_tile:
                    half = chunk % 2
                    out_ap = g_ps[64 * half:64 * half + sdp, :]
                    nc.tensor.matmul(
                        out_ap, wu, wu,
                        start=(chunk < 2),
                        stop=(chunk >= n_chunks - 2),
                        tile_position=(0, 64 * half),
                        skip_group_check=True,
                    )
                else:
                    nc.tensor.matmul(
                        g_ps[:], wu, wu,
                        start=(chunk == 0),
                        stop=(chunk == n_chunks - 1),
                    )
                chunk += 1

        # Epilogue: partial = sum(G^2) - 4 * sum(B^2), B = G[0:S, S:SD]
        g2 = ep.tile([sdp, sdp], f32, tag="g2")
        g_sb = ep.tile([sdp, sdp], f32, tag="gsb")
        if fwl:
            # shift gB rows [44:88] down to [0:44]: PSUM reads need 32-aligned
            # partition bases, so copy the aligned [32:96] window, then the
            # SBUF->SBUF DMA (no alignment limits) extracts [44:88].
            o_sb = ep.tile([96, SD], f32, tag="osb")
            nc.vector.tensor_copy(o_sb[0:96, :], gb_ps[0:96, :])
            shifted = ep.tile([SD, SD], f32, tag="sh")
            nc.sync.dma_start(out=shifted[:], in_=o_sb[SD:2 * SD, :])
            nc.vector.tensor_add(g_sb[:], ga_ps[0:SD, :], shifted[:])
        elif col_tile:
            # DVE lanes can't read across partition bases, so shift the odd
            # half down with a tiny SBUF->SBUF HWDGE DMA and add the halves.
            o_sb = ep.tile([64 + sdp, sdp], f32, tag="osb")
            nc.vector.tensor_copy(o_sb[64:64 + sdp, :], g_ps[64:64 + sdp, :])
            shifted = ep.tile([sdp, sdp], f32, tag="sh")
            nc.sync.dma_start(out=shifted[:], in_=o_sb[64:64 + sdp, :])
            nc.vector.tensor_add(g_sb[:], g_ps[0:sdp, :], shifted[:])
        else:
            nc.vector.tensor_copy(g_sb[:], g_ps[0:sdp, :])
        nc.vector.tensor_mul(g2[:], g_sb[:], g_sb[:])
        colsum = ep.tile([sdp, 1], f32, tag="cs")
        nc.vector.reduce_sum(colsum[:], g2[:], axis=mybir.AxisListType.X)
        bcol = ep.tile([S, 1], f32, tag="bc")
        nc.vector.reduce_sum(bcol[:], g2[0:S, S:SD], axis=mybir.AxisListType.X)
        bneg = ep.tile([S, 1], f32, tag="bn")
        nc.vector.tensor_scalar_mul(bneg[:], bcol[:], -4.0)
        s_ps = psum_pool.tile([1, 1], f32, tag="s")
        nc.tensor.matmul(s_ps[:], colsum[:], ones[:], start=True, stop=False)
        nc.tensor.matmul(s_ps[:], bneg[:], ones[0:S, :], start=False, stop=True)
        res = ep.tile([1, 1], f32, tag="r")
        nc.vector.tensor_copy(res[:], s_ps[:])
        nc.sync.dma_start(out=OUT[:, :], in_=res[:])

    nc.finalize()
    return nc


def _get_nc():
    key = (MODE, COL_TILE, W_BUFS)
    if key not in _nc_cache:
        _nc_cache[key] = _build_nc(key)
    return _nc_cache[key]


def _host_w(embeddings, assignments):
    _, np_dt = _mode_dt(MODE)
    sdp = SD
    ws = []
    for i in range(N_CORES):
        w = np.zeros((N, sdp), dtype=np_dt)
        w[:, 0:S] = assignments[i].reshape(N, S).astype(np_dt)
        w[:, S:SD] = embeddings[i].reshape(N, D).astype(np_dt)
        ws.append(w)
    return ws


def _run(embeddings: np.ndarray, assignments: np.ndarray, trace: bool = False):
    nc = _get_nc()
    in_maps = [{"w": w} for w in _host_w(embeddings, assignments)]
    try:
        res = run_bass_kernel_spmd(
            nc, in_maps, core_ids=list(range(N_CORES)), trace=trace
        )
    except Exception:
        # transient NRT/device hiccups (e.g. NRT_EXEC_UNIT_UNRECOVERABLE)
        # have been observed to succeed on retry
        res = run_bass_kernel_spmd(
            nc, in_maps, core_ids=list(range(N_CORES)), trace=trace
        )
    partials = [float(r["partial"][0, 0]) for r in res.results]
    total = np.float32(np.sum(np.asarray(partials, dtype=np.float64)) / (B * N))
    return np.asarray(total, dtype=np.float32), res


def kernel(embeddings: np.ndarray, assignments: np.ndarray) -> np.ndarray:
    out, _ = _run(embeddings, assignments, trace=False)
    return out


# revision 13
# speedup vs baseline: 1.4413x; 1.3634x over previous
"""DeepClusteringLoss Trainium2 kernel.

loss = (||V^T V||_F^2 - 2 ||V^T E||_F^2 + ||E^T E||_F^2) / (B*N)
summed over batch, with E = embeddings.reshape(B, N, D), V =
assignments.reshape(B, N, S), N = F*T.

Sharding: data-parallel over batch; each of the 8 cores handles one batch
element.  The host pre-interleaves W = [V | E] (N x 44) and pre-casts to a
narrow dtype (fp8e4m3 by default), so the on-chip work is a pure Gram
accumulation G = W^T W streamed through the PE array: per 128-row chunk one
LDWEIGHTS+MATMUL pair (or per 256 rows in fp8 DoubleRow mode), accumulating
in PSUM fp32.  Host casting is mathematically identical to casting on-chip
(the matmul operands are narrow either way) but halves/quarters the
compulsory HBM reads, which is the roofline for this kernel.

All DMAs are HWDGE, issued alternately from the Sync and Scalar queues so
descriptor generation never serializes behind one engine.  There are no
on-chip interleave copies: the DMA'd tile is matmul-ready.  The per-core
scalar partial loss = ||G||^2 - 4 ||B||^2 (B = V^T E block) is reduced
on-device; the host sums the 8 partials and divides by B*N.
"""

import os
from contextlib import ExitStack

import numpy as np
import ml_dtypes

import concourse.bacc as bacc
import concourse.mybir as mybir
import concourse.tile as tile
from concourse.bass_utils import run_bass_kernel_spmd

B, F, T, D, S = 8, 256, 512, 40, 4
N = F * T              # rows per core (131072)
SD = S + D             # 44 combined features
P = 128                # partitions / chunk rows
N_CORES = 8

# MODE: fp16 | fp8 | fp8fwl
# fp8fwl: stationary spans 128 columns (two 44-col chunks + 40 junk cols)
# so the compiler's Fast Weight Load kicks in (NumWeights==128), loading
# weights at 4 fp8/cycle instead of 1; the junk contributions land in PSUM
# partitions the epilogue never reads.
MODE = os.environ.get("KERNEL_MODE", "fp8fwl")
COL_TILE = os.environ.get("KERNEL_COL_TILE", "0") == "1"
W_BUFS = int(os.environ.get("KERNEL_BUFS", "6"))

# block schedule in 128-row chunks: small blocks first for fast engine
# start-up, small blocks last so the final DMA->matmul drain is short.
BLOCKS = [32, 64] + [128] * 6 + [96, 32, 16, 8, 8]
assert sum(BLOCKS) == N // P

_nc_cache = {}


def _mode_dt(mode):
    if mode == "fp16":
        return mybir.dt.float16, np.float16
    return mybir.dt.float8e4, ml_dtypes.float8_e4m3


def _build_nc(key):
    mode, col_tile, w_bufs = key
    mm_dt, _ = _mode_dt(mode)
    f32 = mybir.dt.float32
    fwl = mode == "fp8fwl"
    sdp = SD

    nc = bacc.Bacc("TRN2", target_bir_lowering=False, debug=False)
    W = nc.dram_tensor("w", (N, sdp), mm_dt, kind="ExternalInput")
    OUT = nc.dram_tensor("partial", (1, 1), f32, kind="ExternalOutput")

    with tile.TileContext(nc) as tc, ExitStack() as ctx:
        # Whole W fits in SBUF (44-48 KB/partition in fp8), so every block
        # gets its own persistent tile: matmuls never wait on buffer
        # recycling, and the DMA stream runs purely engine-rate-limited.
        w_pool = ctx.enter_context(tc.tile_pool(name="w", bufs=1))
        psum_pool = ctx.enter_context(tc.tile_pool(name="ps", bufs=1, space="PSUM"))
        # col_tile: even chunks accumulate into partitions [0:SD] (PE col
        # groups 0-1), odd chunks into [64:64+SD] (col groups 2-3)
        g_ps = psum_pool.tile([64 + sdp if col_tile else sdp, sdp], f32, tag="g")

        ep = ctx.enter_context(tc.tile_pool(name="ep", bufs=1))
        ones = ep.tile([sdp, 1], f32, tag="on")
        nc.vector.memset(ones[:], 1.0)

        if fwl:
            # one PSUM accumulator [128, 88]: even-chunk Grams accumulate at
            # [0:44, 0:44], odd-chunk Grams at [44:88, 44:88], cross products
            # land in the junk quadrants and are never read
            gf_ps = psum_pool.tile([P, 2 * SD], f32, tag="gf")

        chunk = 0          # global chunk counter
        n_chunks = sum(BLOCKS)
        n_pairs = n_chunks // 2
        pair = 0
        r0 = 0
        for blk, ub in enumerate(BLOCKS):
            rows = ub * P
            w_ap = W[r0:r0 + rows, :].rearrange("(p u) c -> p (u c)", p=P)
            r0 += rows
            w_t = w_pool.tile([P, ub * sdp], mm_dt, tag=f"w{blk}")
            # alternate the two HWDGE queues so descriptor-gen is parallel
            eng = nc.sync if blk % 2 == 0 else nc.scalar
            eng.dma_start(out=w_t[:], in_=w_ap)

            w3 = w_t[:].rearrange("p (u c) -> p u c", c=sdp)
            if fwl:
                for u in range(0, ub, 2):
                    # 128-col stationary enables FWL (4 fp8/cycle weight
                    # load); the last pair of each block falls back to 88
                    # cols so the junk cols don't read past the tile
                    wcols = 2 * sdp if u == ub - 2 else 128
                    wst = w_t[0:P, u * sdp:u * sdp + wcols]
                    mv = w_t[0:P, u * sdp:(u + 2) * sdp]   # [128, 88] moving
                    nc.tensor.matmul(
                        gf_ps[0:wcols, :], wst, mv,
                        start=(pair == 0), stop=(pair == n_pairs - 1),
                        skip_group_check=True,
                    )
                    pair += 1
                continue
            for u in range(ub):
                wu = w3[:, u, :]
                if col_tile:
                    half = chunk % 2
                    out_ap = g_ps[64 * half:64 * half + sdp, :]
                    nc.tensor.matmul(
                        out_ap, wu, wu,
                        start=(chunk < 2),
                        stop=(chunk >= n_chunks - 2),
                        tile_position=(0, 64 * half),
                        skip_group_check=True,
                    )
                else:
                    nc.tensor.matmul(
                        g_ps[:], wu, wu,
                        start=(chunk == 0),
                        stop=(chunk == n_chunks - 1),
                    )
                chunk += 1

        # Epilogue: partial = sum(G^2) - 4 * sum(B^2), B = G[0:S, S:SD]
        g2 = ep.tile([sdp, sdp], f32, tag="g2")
        g_sb = ep.tile([sdp, sdp], f32, tag="gsb")
        if fwl:
            # extract the two diagonal 44x44 blocks of the [88, 88] result:
            # copy PSUM->SBUF, shift rows/cols [44:88] down with a SBUF->SBUF
            # DMA (no partition-alignment limits), and add.
            o_sb = ep.tile([2 * SD, 2 * SD], f32, tag="osb")
            nc.vector.tensor_copy(o_sb[:], gf_ps[0:2 * SD, :])
            shifted = ep.tile([SD, SD], f32, tag="sh")
            nc.sync.dma_start(out=shifted[:], in_=o_sb[SD:2 * SD, SD:2 * SD])
            nc.vector.tensor_add(g_sb[:], o_sb[0:SD, 0:SD], shifted[:])
        elif col_tile:
            # DVE lanes can't read across partition bases, so shift the odd
            # half down with a tiny SBUF->SBUF HWDGE DMA and add the halves.
            o_sb = ep.tile([64 + sdp, sdp], f32, tag="osb")
            nc.vector.tensor_copy(o_sb[64:64 + sdp, :], g_ps[64:64 + sdp, :])
            shifted = ep.tile([sdp, sdp], f32, tag="sh")
            nc.sync.dma_start(out=shifted[:], in_=o_sb[64:64 + sdp, :])
            nc.vector.tensor_add(g_sb[:], g_ps[0:sdp, :], shifted[:])
        else:
            nc.vector.tensor_copy(g_sb[:], g_ps[0:sdp, :])
        nc.vector.tensor_mul(g2[:], g_sb[:], g_sb[:])
        colsum = ep.tile([sdp, 1], f32, tag="cs")
        nc.vector.reduce_sum(colsum[:], g2[:], axis=mybir.AxisListType.X)
        bcol = ep.tile([S, 1], f32, tag="bc")
        nc.vector.reduce_sum(bcol[:], g2[0:S, S:SD], axis=mybir.AxisListType.X)
        bneg = ep.tile([S, 1], f32, tag="bn")
        nc.vector.tensor_scalar_mul(bneg[:], bcol[:], -4.0)
        s_ps = psum_pool.tile([1, 1], f32, tag="s")
        nc.tensor.matmul(s_ps[:], colsum[:], ones[:], start=True, stop=False)
        nc.tensor.matmul(s_ps[:], bneg[:], ones[0:S, :], start=False, stop=True)
        res = ep.tile([1, 1], f32, tag="r")
        nc.vector.tensor_copy(res[:], s_ps[:])
        nc.sync.dma_start(out=OUT[:, :], in_=res[:])

    nc.finalize()
    return nc


def _get_nc():
    key = (MODE, COL_TILE, W_BUFS)
    if key not in _nc_cache:
        _nc_cache[key] = _build_nc(key)
    return _nc_cache[key]


def _host_w(embeddings, assignments):
    _, np_dt = _mode_dt(MODE)
    sdp = SD
    ws = []
    for i in range(N_CORES):
        w = np.zeros((N, sdp), dtype=np_dt)
        w[:, 0:S] = assignments[i].reshape(N, S).astype(np_dt)
        w[:, S:SD] = embeddings[i].reshape(N, D).astype(np_dt)
        ws.append(w)
    return ws


def _run(embeddings: np.ndarray, assignments: np.ndarray, trace: bool = False):
    nc = _get_nc()
    in_maps = [{"w": w} for w in _host_w(embeddings, assignments)]
    try:
        res = run_bass_kernel_spmd(
            nc, in_maps, core_ids=list(range(N_CORES)), trace=trace
        )
    except Exception:
        # transient NRT/device hiccups (e.g. NRT_EXEC_UNIT_UNRECOVERABLE)
        # have been observed to succeed on retry
        res = run_bass_kernel_spmd(
            nc, in_maps, core_ids=list(range(N_CORES)), trace=trace
        )
    partials = [float(r["partial"][0, 0]) for r in res.results]
    total = np.float32(np.sum(np.asarray(partials, dtype=np.float64)) / (B * N))
    return np.asarray(total, dtype=np.float32), res


def kernel(embeddings: np.ndarray, assignments: np.ndarray) -> np.ndarray:
    out, _ = _run(embeddings, assignments, trace=False)
    return out
